# revision 1
# baseline (speedup 1.0000x reference)
"""Trainium2 Bass kernel for nn_Criterion_32830730011569.

Strategy: 8 cores = (image b in 0..3) x (H-half h in 0..1). Each core streams
its [96,192] pixel slice of the big tensors from HBM once:
  - dice: per-pixel softmax over matched portion channels is reformulated so no
    channel gather is needed in the hot loop:
      num_b = 2 * sum_m C[me[m], mq[m]],  C = sum_pixels (true/Z)^T (exp(por)*mask)
    accumulated as bf16 matmuls into one [96,160] PSUM tile; den_b = sum(true) +
    H*W (softmax sums to 1) with the constant added on host.
  - occupancy CE: streamed logsumexp + label-select.
  - 7x7-window BCE: indirect DMAs gather window rows (one offset/partition);
    each half-core sums only the window pixels that live in its slice.
  - class / NLL: tiny one-hot matmul gathers.
Each core returns 7 partial sums; the host combines them into the scalar loss.
"""
import sys

sys.path.insert(0, "/opt/trn_rl_repo")
import numpy as np

B, H, W, Q, E, M, K, WIN = 4, 192, 192, 160, 96, 96, 4, 7
NO_E = 0.1
HALF = H // 2          # rows per core slice
NPIX = HALF * W        # 18432 pixels per slice
P = 128                # partitions
J = NPIX // P          # 144 pixels per partition (p-major)
NCHUNK = 8
JC = J // NCHUNK       # 18
MAGIC = 8388608.0      # 2^23
MAGIC_I = 0x4B000000

_CACHE = {}


def _build_nc():
    import os
    import concourse.bass as bass
    import concourse.bacc as bacc
    import concourse.tile as tile
    from concourse import mybir

    DIS = set(os.environ.get("KDIS", "").split(","))

    f32 = mybir.dt.float32
    i32 = mybir.dt.int32
    bf16 = mybir.dt.bfloat16
    AF = mybir.ActivationFunctionType
    OP = mybir.AluOpType
    AX = mybir.AxisListType

    nc = bacc.Bacc("TRN2", target_bir_lowering=False, debug=False, num_devices=8)

    # ---- external I/O ----
    true_sl = nc.dram_tensor("true_sl", [NPIX, E], f32, kind="ExternalInput")
    por_sl = nc.dram_tensor("por_sl", [NPIX, Q], f32, kind="ExternalInput")
    bin_sl = nc.dram_tensor("bin_sl", [NPIX, Q], f32, kind="ExternalInput")
    occ_sl = nc.dram_tensor("occ_sl", [NPIX, K], f32, kind="ExternalInput")
    occt_f = nc.dram_tensor("occt_f", [P, J], f32, kind="ExternalInput")
    me_colf = nc.dram_tensor("me_colf", [M, 1], f32, kind="ExternalInput")
    mq_colf = nc.dram_tensor("mq_colf", [M, 1], f32, kind="ExternalInput")
    me_row_rep = nc.dram_tensor("me_row_rep", [E, M], f32, kind="ExternalInput")
    mq_row_rep_a = nc.dram_tensor("mq_row_rep_a", [P, M], f32, kind="ExternalInput")
    mq_row_rep_b = nc.dram_tensor("mq_row_rep_b", [Q - P, M], f32, kind="ExternalInput")
    iota_q_row = nc.dram_tensor("iota_q_row", [M, Q], f32, kind="ExternalInput")
    iota_e_row = nc.dram_tensor("iota_e_row", [M, E], f32, kind="ExternalInput")
    iota_p_a = nc.dram_tensor("iota_p_a", [P, 1], f32, kind="ExternalInput")
    iota_p_b = nc.dram_tensor("iota_p_b", [Q - P, 1], f32, kind="ExternalInput")
    iota_p_96 = nc.dram_tensor("iota_p_96", [E, 1], f32, kind="ExternalInput")
    drof_rep = nc.dram_tensor("drof_rep", [M, WIN], f32, kind="ExternalInput")
    inc_pts = nc.dram_tensor("inc_pts", [E, 2], f32, kind="ExternalInput")
    pos_a = nc.dram_tensor("pos_a", [P, 2], f32, kind="ExternalInput")
    pos_b = nc.dram_tensor("pos_b", [Q - P, 2], f32, kind="ExternalInput")
    chol_a = nc.dram_tensor("chol_a", [P, 4], f32, kind="ExternalInput")
    chol_b = nc.dram_tensor("chol_b", [Q - P, 4], f32, kind="ExternalInput")
    iel_row = nc.dram_tensor("iel_row", [1, Q], f32, kind="ExternalInput")
    rb_neg = nc.dram_tensor("rb_neg", [M, 1], f32, kind="ExternalInput")
    partials = nc.dram_tensor("partials", [1, 8], f32, kind="ExternalOutput")

    def bc(ap, pos, count):
        """Insert a stride-0 broadcast dim into an AP at free-dim position pos."""
        new = list(ap.ap)
        new.insert(pos, [0, count])
        return bass.AP(tensor=ap.tensor, offset=ap.offset, ap=new)

    from contextlib import ExitStack

    with tile.TileContext(nc) as tc, ExitStack() as ctx:
        sing = ctx.enter_context(tc.tile_pool(name="sing", bufs=1))
        big = ctx.enter_context(tc.tile_pool(name="big", bufs=2))
        ps = ctx.enter_context(tc.tile_pool(name="ps", bufs=1, space="PSUM"))

        # ---------- small loads ----------
        def load(dram, shape):
            nm = dram.name + "_sb"
            t = sing.tile(shape, f32, name=nm, tag=nm)
            nc.sync.dma_start(out=t[:], in_=dram.ap())
            return t

        me_c = load(me_colf, [M, 1])
        mq_c = load(mq_colf, [M, 1])
        me_rr = load(me_row_rep, [E, M])
        mq_rra = load(mq_row_rep_a, [P, M])
        mq_rrb = load(mq_row_rep_b, [Q - P, M])
        io_q = load(iota_q_row, [M, Q])
        io_e = load(iota_e_row, [M, E])
        io_pa = load(iota_p_a, [P, 1])
        io_pb = load(iota_p_b, [Q - P, 1])
        io_p96 = load(iota_p_96, [E, 1])
        drof = load(drof_rep, [M, WIN])
        inc_sb = load(inc_pts, [E, 2])
        posa = load(pos_a, [P, 2])
        posb = load(pos_b, [Q - P, 2])
        chola = load(chol_a, [P, 4])
        cholb = load(chol_b, [Q - P, 4])
        iel = load(iel_row, [1, Q])
        rbn = load(rb_neg, [M, 1])

        def emit_ln(pref, out, x, pp, ff):
            """out = ln(x) for positive normal floats.

            Bit-extract exponent/mantissa (no float<->int casts needed), 3-term
            series on the reduced mantissa, then 2 Newton steps y += x*e^-y - 1
            using the ACT Exp table.
            """
            LN2 = 0.6931471805599453
            SQRT2 = 1.4142135623730951

            def T(nm, dt=f32):
                return sing.tile([pp, ff], dt, name=f"{pref}_{nm}", tag=f"{pref}_{nm}")

            xb = x.bitcast(i32)
            ei = T("ei", i32)
            nc.vector.tensor_scalar(out=ei[:], in0=xb, scalar1=23, scalar2=MAGIC_I,
                                    op0=OP.arith_shift_right, op1=OP.bitwise_or)
            ef = T("ef")
            nc.vector.tensor_scalar(out=ef[:], in0=ei[:].bitcast(f32),
                                    scalar1=-(MAGIC + 127.0), scalar2=None, op0=OP.add)
            mi = T("mi", i32)
            nc.vector.tensor_scalar(out=mi[:], in0=xb, scalar1=0x007FFFFF,
                                    scalar2=0x3F800000, op0=OP.bitwise_and, op1=OP.bitwise_or)
            mf = mi[:].bitcast(f32)
            cf = T("cf")
            nc.vector.tensor_scalar(out=cf[:], in0=mf, scalar1=SQRT2, scalar2=None, op0=OP.is_ge)
            hf = T("hf")
            nc.vector.tensor_scalar(out=hf[:], in0=cf[:], scalar1=-0.5, scalar2=1.0,
                                    op0=OP.mult, op1=OP.add)
            u = T("u")
            nc.vector.tensor_tensor(out=u[:], in0=mf, in1=hf[:], op=OP.mult)
            nc.vector.tensor_tensor(out=ef[:], in0=ef[:], in1=cf[:], op=OP.add)
            nc.vector.tensor_scalar(out=u[:], in0=u[:], scalar1=-1.0, scalar2=None, op0=OP.add)
            v = T("v")
            nc.vector.tensor_scalar(out=v[:], in0=u[:], scalar1=-0.25, scalar2=1.0 / 3.0,
                                    op0=OP.mult, op1=OP.add)
            nc.vector.tensor_tensor(out=v[:], in0=v[:], in1=u[:], op=OP.mult)
            nc.vector.tensor_scalar(out=v[:], in0=v[:], scalar1=-0.5, scalar2=None, op0=OP.add)
            nc.vector.tensor_tensor(out=v[:], in0=v[:], in1=u[:], op=OP.mult)
            nc.vector.tensor_scalar(out=v[:], in0=v[:], scalar1=1.0, scalar2=None, op0=OP.add)
            nc.vector.tensor_tensor(out=v[:], in0=v[:], in1=u[:], op=OP.mult)
            y = out
            nc.vector.tensor_scalar(out=y, in0=ef[:], scalar1=LN2, scalar2=None, op0=OP.mult)
            nc.vector.tensor_tensor(out=y, in0=y, in1=v[:], op=OP.add)
            ey = T("ey")
            w = T("w")
            for _ in range(2):
                nc.scalar.activation(out=ey[:], in_=y, func=AF.Exp, scale=-1.0)
                nc.vector.tensor_tensor(out=w[:], in0=ey[:], in1=x, op=OP.mult)
                nc.vector.tensor_scalar(out=w[:], in0=w[:], scalar1=-1.0, scalar2=None, op0=OP.add)
                nc.vector.tensor_tensor(out=y, in0=y, in1=w[:], op=OP.add)

        def emit_softplus(pref, out, x, pp, ff):
            """out = ln(1 + exp(x)) (inputs are O(1) logits, no overflow)."""
            opx = sing.tile([pp, ff], f32, name=pref + "_opx", tag=pref + "_opx")
            nc.scalar.activation(out=opx[:], in_=x, func=AF.Exp)
            nc.vector.tensor_scalar(out=opx[:], in0=opx[:], scalar1=1.0, scalar2=None, op0=OP.add)
            emit_ln(pref, out, opx[:], pp, ff)

        ones = sing.tile([P, 1], f32)
        nc.vector.memset(ones[:], 1.0)
        onesw = sing.tile([E, P], f32)
        nc.vector.memset(onesw[:], 1.0)

        stats = sing.tile([P, 6], f32)
        nc.vector.memset(stats[:], 0.0)
        res = sing.tile([1, 8], f32)
        nc.vector.memset(res[:], 0.0)

        # ---------- one-hot selectors ----------
        Mq = sing.tile([M, Q], f32)
        nc.vector.tensor_scalar(out=Mq[:], in0=io_q[:], scalar1=mq_c[:], scalar2=None, op0=OP.is_equal)
        Me = sing.tile([M, E], f32)
        nc.vector.tensor_scalar(out=Me[:], in0=io_e[:], scalar1=me_c[:], scalar2=None, op0=OP.is_equal)
        MeT = sing.tile([E, M], f32)
        nc.vector.tensor_scalar(out=MeT[:], in0=me_rr[:], scalar1=io_p96[:], scalar2=None, op0=OP.is_equal)
        MqTa = sing.tile([P, M], f32)
        nc.vector.tensor_scalar(out=MqTa[:], in0=mq_rra[:], scalar1=io_pa[:], scalar2=None, op0=OP.is_equal)
        MqTb = sing.tile([Q - P, M], f32)
        nc.vector.tensor_scalar(out=MqTb[:], in0=mq_rrb[:], scalar1=io_pb[:], scalar2=None, op0=OP.is_equal)

        # ---------- tiny matmul gathers ----------
        pts_ps = ps.tile([M, 2], f32)
        nc.tensor.matmul(out=pts_ps[:], lhsT=MeT[:], rhs=inc_sb[:], start=True, stop=True)
        ptsr = sing.tile([M, 2], f32)
        nc.vector.tensor_copy(out=ptsr[:], in_=pts_ps[:])

        cen_ps = ps.tile([M, 2], f32)
        nc.tensor.matmul(out=cen_ps[:], lhsT=MqTa[:], rhs=posa[:], start=True, stop=False)
        nc.tensor.matmul(out=cen_ps[:], lhsT=MqTb[:], rhs=posb[:], start=False, stop=True)
        cenr = sing.tile([M, 2], f32)
        nc.vector.tensor_copy(out=cenr[:], in_=cen_ps[:])

        chr_ps = ps.tile([M, 4], f32)
        nc.tensor.matmul(out=chr_ps[:], lhsT=MqTa[:], rhs=chola[:], start=True, stop=False)
        nc.tensor.matmul(out=chr_ps[:], lhsT=MqTb[:], rhs=cholb[:], start=False, stop=True)
        cholr = sing.tile([M, 4], f32)
        nc.vector.tensor_copy(out=cholr[:], in_=chr_ps[:])

        H_ps = ps.tile([E, Q], f32)
        nc.tensor.matmul(out=H_ps[:], lhsT=Me[:], rhs=Mq[:], start=True, stop=True)
        Hs = sing.tile([E, Q], f32)
        nc.vector.tensor_copy(out=Hs[:], in_=H_ps[:])

        # matched-q indicator, replicated to all partitions (column sums of Mq)
        ind_ps = ps.tile([P, Q], f32)
        nc.tensor.matmul(out=ind_ps[:], lhsT=onesw[:], rhs=Mq[:], start=True, stop=True)
        ind_bf = sing.tile([P, Q], bf16)
        nc.vector.tensor_copy(out=ind_bf[:], in_=ind_ps[:])
        ind1 = sing.tile([1, Q], f32)
        nc.vector.tensor_copy(out=ind1[:], in_=ind_ps[0:1, :])

        # ---------- window offsets ----------
        rmag = sing.tile([M, 2], f32)
        nc.vector.tensor_scalar(out=rmag[:], in0=ptsr[:], scalar1=MAGIC, scalar2=-MAGIC,
                                op0=OP.add, op1=OP.add)
        gtm = sing.tile([M, 2], f32)
        nc.vector.tensor_tensor(out=gtm[:], in0=rmag[:], in1=ptsr[:], op=OP.is_gt)
        pixf = sing.tile([M, 2], f32)
        nc.vector.tensor_tensor(out=pixf[:], in0=rmag[:], in1=gtm[:], op=OP.subtract)
        base = sing.tile([M, 1], f32)
        nc.vector.tensor_scalar(out=base[:], in0=pixf[:, 0:1], scalar1=float(W),
                                scalar2=float(-3 * W - 3), op0=OP.mult, op1=OP.add)
        nc.vector.tensor_tensor(out=base[:], in0=base[:], in1=pixf[:, 1:2], op=OP.add)
        sofs = sing.tile([M, WIN], f32)
        nc.vector.tensor_scalar(out=sofs[:], in0=drof[:], scalar1=base[:], scalar2=rbn[:],
                                op0=OP.add, op1=OP.add)
        v1 = sing.tile([M, WIN], f32)
        nc.vector.tensor_scalar(out=v1[:], in0=sofs[:], scalar1=0.0, scalar2=None, op0=OP.is_ge)
        v2 = sing.tile([M, WIN], f32)
        nc.vector.tensor_scalar(out=v2[:], in0=sofs[:], scalar1=float(NPIX - 1), scalar2=None, op0=OP.is_le)
        valid = sing.tile([M, WIN], f32)
        nc.vector.tensor_tensor(out=valid[:], in0=v1[:], in1=v2[:], op=OP.mult)
        clam = sing.tile([M, WIN], f32)
        nc.vector.tensor_scalar(out=clam[:], in0=sofs[:], scalar1=0.0, scalar2=float(NPIX - WIN),
                                op0=OP.max, op1=OP.min)
        # element offsets: clamped_flat_pixel * C + matched channel (< 2^23, exact)
        soft = sing.tile([M, WIN], f32)
        nc.vector.tensor_scalar(out=soft[:], in0=clam[:], scalar1=float(E), scalar2=me_c[:],
                                op0=OP.mult, op1=OP.add)
        nc.vector.tensor_scalar(out=soft[:], in0=soft[:], scalar1=MAGIC, scalar2=None, op0=OP.add)
        soft_i = sing.tile([M, WIN], i32)
        nc.vector.tensor_scalar(out=soft_i[:], in0=soft[:].bitcast(i32), scalar1=0x007FFFFF,
                                scalar2=None, op0=OP.bitwise_and)
        sofb = sing.tile([M, WIN], f32)
        nc.vector.tensor_scalar(out=sofb[:], in0=clam[:], scalar1=float(Q), scalar2=mq_c[:],
                                op0=OP.mult, op1=OP.add)
        nc.vector.tensor_scalar(out=sofb[:], in0=sofb[:], scalar1=MAGIC, scalar2=None, op0=OP.add)
        sofb_i = sing.tile([M, WIN], i32)
        nc.vector.tensor_scalar(out=sofb_i[:], in0=sofb[:].bitcast(i32), scalar1=0x007FFFFF,
                                scalar2=None, op0=OP.bitwise_and)

        # ---------- window gathers (one indirect DMA per window row) ----------
        WINDOWS_ON = "win" not in DIS
        RUNT = (WIN - 1) * E + 1
        RUNB = (WIN - 1) * Q + 1
        tw = sing.tile([M, WIN, RUNT], f32)
        bw = sing.tile([M, WIN, RUNB], f32)
        true_flat = bass.AP(tensor=true_sl.ap().tensor, offset=0, ap=[[1, NPIX * E], [1, 1]])
        bin_flat = bass.AP(tensor=bin_sl.ap().tensor, offset=0, ap=[[1, NPIX * Q], [1, 1]])
        for dr in range(WIN if WINDOWS_ON else 0):
            nc.gpsimd.indirect_dma_start(
                out=tw[:, dr, :], out_offset=None, in_=true_flat,
                in_offset=bass.IndirectOffsetOnAxis(ap=soft_i[:, dr:dr + 1], axis=0))
            nc.gpsimd.indirect_dma_start(
                out=bw[:, dr, :], out_offset=None, in_=bin_flat,
                in_offset=bass.IndirectOffsetOnAxis(ap=sofb_i[:, dr:dr + 1], axis=0))

        if not WINDOWS_ON:
            nc.vector.memset(tw[:], 0.0)
            nc.vector.memset(bw[:], 0.0)
        # ---------- dice streaming ----------
        por_v = por_sl.ap().rearrange("(p j) q -> p j q", p=P)
        true_v = true_sl.ap().rearrange("(p j) e -> p j e", p=P)
        C_ps = ps.tile([E, Q], f32)
        for c in range(NCHUNK):
            sl = slice(c * JC, (c + 1) * JC)
            por_t = big.tile([P, JC, Q], f32, tag="por")
            nc.sync.dma_start(out=por_t[:], in_=por_v[:, sl, :])
            true_t = big.tile([P, JC, E], f32, tag="true")
            nc.sync.dma_start(out=true_t[:], in_=true_v[:, sl, :])
            exp_t = big.tile([P, JC, Q], bf16, tag="exp")
            nc.scalar.activation(out=exp_t[:], in_=por_t[:], func=AF.Exp)
            nc.vector.tensor_tensor(out=exp_t[:], in0=exp_t[:], in1=bc(ind_bf[:], 1, JC), op=OP.mult)
            z_t = big.tile([P, JC], f32, tag="z")
            z_eng = nc.gpsimd if (os.environ.get("GPZ") and c % 2 == 1) else nc.vector
            z_eng.reduce_sum(out=z_t[:], in_=exp_t[:], axis=AX.X)
            rz_t = big.tile([P, JC], f32, tag="rz")
            nc.vector.reciprocal(out=rz_t[:], in_=z_t[:])
            a_t = big.tile([P, JC, E], bf16, tag="a")
            a_inst = nc.vector.tensor_tensor(out=a_t[:], in0=true_t[:], in1=bc(rz_t[:], 2, E), op=OP.mult)
            if c == NCHUNK - 1:
                last_dice_dve = a_inst
            for kb in range(JC if "mm" not in DIS else 0):
                nc.tensor.matmul(out=C_ps[:], lhsT=a_t[:, kb, :], rhs=exp_t[:, kb, :],
                                 start=(c == 0 and kb == 0),
                                 stop=(c == NCHUNK - 1 and kb == JC - 1))
        if "mm" in DIS:
            nc.tensor.matmul(out=C_ps[:], lhsT=a_t[:, 0, :], rhs=exp_t[:, 0, :],
                             start=True, stop=True)

        Cs = sing.tile([E, Q], f32)
        nc.vector.tensor_copy(out=Cs[:], in_=C_ps[:])
        # C's rhs was masked exp, so sum_q C[e,q] = sum_pixels true[p,e] (the
        # 1/Z in the stationary cancels the masked-exp row sums): den for free.
        nc.vector.reduce_sum(out=stats[0:E, 3:4], in_=Cs[:], axis=AX.X)
        scr_c = sing.tile([E, Q], f32)
        nc.vector.tensor_tensor(out=scr_c[:], in0=Cs[:], in1=Hs[:], op=OP.mult)
        nc.vector.reduce_sum(out=stats[0:M, 2:3], in_=scr_c[:], axis=AX.X)

        # ---------- occupancy CE ----------
        occ_v = occ_sl.ap().rearrange("(p j) k -> p j k", p=P)
        occ_t = sing.tile([P, J, K], f32)
        nc.sync.dma_start(out=occ_t[:], in_=occ_v)
        oct_t = sing.tile([P, J], f32)
        nc.sync.dma_start(out=oct_t[:], in_=occt_f.ap())
        e4 = sing.tile([P, J, K], f32)
        nc.scalar.activation(out=e4[:], in_=occ_t[:], func=AF.Exp)
        s4 = sing.tile([P, J], f32)
        nc.vector.reduce_sum(out=s4[:], in_=e4[:], axis=AX.X)
        lse = sing.tile([P, J], f32)
        emit_ln("occ", lse[:], s4[:], P, J)
        xt = sing.tile([P, J], f32)
        mk = sing.tile([P, J], f32)
        pk = sing.tile([P, J], f32)
        for k in range(K):
            nc.vector.tensor_scalar(out=mk[:], in0=oct_t[:], scalar1=float(k), scalar2=None, op0=OP.is_equal)
            if k == 0:
                nc.vector.tensor_tensor(out=xt[:], in0=mk[:], in1=occ_t[:, :, k], op=OP.mult)
            else:
                nc.vector.tensor_tensor(out=pk[:], in0=mk[:], in1=occ_t[:, :, k], op=OP.mult)
                nc.vector.tensor_tensor(out=xt[:], in0=xt[:], in1=pk[:], op=OP.add)
        nc.vector.tensor_tensor(out=lse[:], in0=lse[:], in1=xt[:], op=OP.subtract)
        nc.vector.reduce_sum(out=stats[:, 4:5], in_=lse[:], axis=AX.X)

        # ---------- class loss (partition 0) ----------
        sp = sing.tile([1, Q], f32)
        emit_softplus("cls", sp[:], iel[:], 1, Q)
        t9 = sing.tile([1, Q], f32)
        nc.vector.tensor_scalar(out=t9[:], in0=sp[:], scalar1=0.9, scalar2=None, op0=OP.mult)
        nc.vector.tensor_tensor(out=t9[:], in0=t9[:], in1=iel[:], op=OP.subtract)
        scr_q = sing.tile([1, Q], f32)
        clsm = sing.tile([1, 1], f32)
        nc.vector.tensor_tensor(out=scr_q[:], in0=t9[:], in1=ind1[:], op=OP.mult)
        nc.vector.reduce_sum(out=clsm[:], in_=scr_q[:], axis=AX.X)
        spsum = sing.tile([1, 1], f32)
        nc.vector.reduce_sum(out=spsum[:], in_=sp[:], axis=AX.X)
        nc.vector.tensor_scalar(out=spsum[:], in0=spsum[:], scalar1=NO_E, scalar2=None, op0=OP.mult)
        nc.vector.tensor_tensor(out=res[:, 6:7], in0=spsum[:], in1=clsm[:], op=OP.add)

        # ---------- NLL (96 partitions) ----------
        d_ = sing.tile([M, 2], f32)
        nc.vector.tensor_tensor(out=d_[:], in0=ptsr[:], in1=cenr[:], op=OP.subtract)
        r00 = sing.tile([M, 1], f32)
        nc.vector.reciprocal(out=r00[:], in_=cholr[:, 0:1])
        r11 = sing.tile([M, 1], f32)
        nc.vector.reciprocal(out=r11[:], in_=cholr[:, 3:4])
        z0 = sing.tile([M, 1], f32)
        nc.vector.tensor_tensor(out=z0[:], in0=d_[:, 0:1], in1=r00[:], op=OP.mult)
        t1 = sing.tile([M, 1], f32)
        nc.vector.tensor_tensor(out=t1[:], in0=cholr[:, 2:3], in1=z0[:], op=OP.mult)
        nc.vector.tensor_tensor(out=t1[:], in0=d_[:, 1:2], in1=t1[:], op=OP.subtract)
        z1 = sing.tile([M, 1], f32)
        nc.vector.tensor_tensor(out=z1[:], in0=t1[:], in1=r11[:], op=OP.mult)
        sq = sing.tile([M, 1], f32)
        nc.vector.tensor_tensor(out=sq[:], in0=z0[:], in1=z0[:], op=OP.mult)
        sq1 = sing.tile([M, 1], f32)
        nc.vector.tensor_tensor(out=sq1[:], in0=z1[:], in1=z1[:], op=OP.mult)
        nc.vector.tensor_tensor(out=sq[:], in0=sq[:], in1=sq1[:], op=OP.add)
        ldet = sing.tile([M, 1], f32)
        nc.vector.tensor_tensor(out=ldet[:], in0=cholr[:, 0:1], in1=cholr[:, 3:4], op=OP.mult)
        lnd = sing.tile([M, 1], f32)
        emit_ln("nld", lnd[:], ldet[:], M, 1)
        nc.vector.tensor_scalar(out=sq[:], in0=sq[:], scalar1=0.5,
                                scalar2=float(np.log(2.0 * np.pi)), op0=OP.mult, op1=OP.add)
        nc.vector.tensor_tensor(out=stats[0:M, 0:1], in0=sq[:], in1=lnd[:], op=OP.add)

        # ---------- window extraction + bce ----------
        def restride_last(ap, step, count):
            new_ap = list(ap.ap)
            new_ap[-1] = [step, count]
            return bass.AP(tensor=ap.tensor, offset=ap.offset, ap=new_ap)

        from concourse.tile import add_dep_helper
        tv = sing.tile([M, WIN * WIN], f32)
        tv_i = nc.vector.tensor_copy(out=tv[:].rearrange("m (a b) -> m a b", a=WIN),
                                     in_=restride_last(tw[:], E, WIN))
        lg = sing.tile([M, WIN * WIN], f32)
        lg_i = nc.vector.tensor_copy(out=lg[:].rearrange("m (a b) -> m a b", a=WIN),
                                     in_=restride_last(bw[:], Q, WIN))
        # keep the gather-dependent extraction out of the dice DVE stream: it
        # must not head-of-line block DVE behind the indirect-DMA drain
        add_dep_helper(tv_i.ins, last_dice_dve.ins, reason="extract after dice")
        add_dep_helper(lg_i.ins, last_dice_dve.ins, reason="extract after dice")
        spw = sing.tile([M, WIN * WIN], f32)
        emit_softplus("win", spw[:], lg[:], M, WIN * WIN)
        prw = sing.tile([M, WIN * WIN], f32)
        nc.vector.tensor_tensor(out=prw[:], in0=lg[:], in1=tv[:], op=OP.mult)
        nc.vector.tensor_tensor(out=spw[:], in0=spw[:], in1=prw[:], op=OP.subtract)
        scr_w = sing.tile([M, WIN * WIN], f32)
        valid49 = sing.tile([M, WIN * WIN], f32)
        nc.vector.tensor_copy(out=valid49[:].rearrange("m (a b) -> m a b", a=WIN),
                              in_=bc(valid[:], 2, WIN))
        nc.vector.tensor_tensor(out=scr_w[:], in0=spw[:], in1=valid49[:], op=OP.mult)
        nc.vector.reduce_sum(out=stats[0:M, 1:2], in_=scr_w[:], axis=AX.X)

        # ---------- final cross-partition reduction ----------
        fin_ps = ps.tile([1, 6], f32)
        nc.tensor.matmul(out=fin_ps[:], lhsT=ones[:], rhs=stats[:], start=True, stop=True)
        nc.vector.tensor_copy(out=res[:, 0:6], in_=fin_ps[:])
        nc.sync.dma_start(out=partials.ap(), in_=res[:])

    nc.compile()
    return nc


def _get_nc():
    if "nc" not in _CACHE:
        _CACHE["nc"] = _build_nc()
    return _CACHE["nc"]


def make_in_maps(is_electron_logit, true_segmap, binary_mask_logits, portion_logits,
                 incidence_points, positions, chol, occupancy_logits, occupancy_true,
                 matched_q, matched_e):
    f = np.float32
    iota_q = np.tile(np.arange(Q, dtype=f), (M, 1))
    iota_e = np.tile(np.arange(E, dtype=f), (M, 1))
    io_pa = np.arange(P, dtype=f).reshape(P, 1)
    io_pb = np.arange(P, Q, dtype=f).reshape(Q - P, 1)
    io_p96 = np.arange(E, dtype=f).reshape(E, 1)
    drof = np.tile((np.arange(WIN, dtype=f) * W), (M, 1))
    in_maps = []
    for c in range(8):
        b, h = c // 2, c % 2
        sl = slice(h * HALF, (h + 1) * HALF)
        me = np.asarray(matched_e[b])
        mq = np.asarray(matched_q[b])
        chol_b = np.asarray(chol[b], dtype=f).reshape(Q, 4)
        pos_b = np.asarray(positions[b], dtype=f)
        in_maps.append(dict(
            true_sl=np.ascontiguousarray(true_segmap[b, sl]).reshape(NPIX, E),
            por_sl=np.ascontiguousarray(portion_logits[b, sl]).reshape(NPIX, Q),
            bin_sl=np.ascontiguousarray(binary_mask_logits[b, sl]).reshape(NPIX, Q),
            occ_sl=np.ascontiguousarray(occupancy_logits[b, sl]).reshape(NPIX, K),
            occt_f=np.ascontiguousarray(occupancy_true[b, sl]).reshape(P, J).astype(f),
            me_colf=me.astype(f).reshape(M, 1),
            mq_colf=mq.astype(f).reshape(M, 1),
            me_row_rep=np.tile(me.astype(f), (E, 1)),
            mq_row_rep_a=np.tile(mq.astype(f), (P, 1)),
            mq_row_rep_b=np.tile(mq.astype(f), (Q - P, 1)),
            iota_q_row=iota_q, iota_e_row=iota_e,
            iota_p_a=io_pa, iota_p_b=io_pb, iota_p_96=io_p96,
            drof_rep=drof,
            inc_pts=np.asarray(incidence_points[b], dtype=f),
            pos_a=pos_b[:P], pos_b=pos_b[P:],
            chol_a=chol_b[:P], chol_b=chol_b[P:],
            iel_row=np.asarray(is_electron_logit, dtype=f).reshape(B, Q)[b].reshape(1, Q),
            rb_neg=np.full((M, 1), -h * NPIX, dtype=f),
        ))
    return in_maps


def combine(partials_list):
    s = np.stack([np.asarray(p, dtype=np.float64).reshape(8) for p in partials_list])
    # slots: 0=nll_sum 1=bce_sum 2=num2_sum 3=den_true_sum 4=occ_sum 6=class_sum
    class_loss = s[0::2, 6].sum() / (B * Q)
    nll_loss = s[0::2, 0].sum() / (B * M)
    bce_loss = s[:, 1].sum() / (B * M * WIN * WIN)
    occ_loss = s[:, 4].sum() / (B * H * W)
    dice = 0.0
    for b in range(B):
        num = 2.0 * (s[2 * b, 2] + s[2 * b + 1, 2])
        den = s[2 * b, 3] + s[2 * b + 1, 3] + H * W
        dice += 1.0 - (num + 1.0) / (den + 1.0)
    dice_loss = dice / B
    return np.float32(class_loss + bce_loss + dice_loss + nll_loss + occ_loss)


def kernel(**inputs):
    from concourse.bass_utils import run_bass_kernel_spmd
    nc = _get_nc()
    in_maps = make_in_maps(**{k: np.asarray(v) for k, v in inputs.items()})
    r = run_bass_kernel_spmd(nc, in_maps, list(range(8)))
    return combine([r.results[c]["partials"] for c in range(8)])



# revision 2
# speedup vs baseline: 1.3437x; 1.3437x over previous
"""Trainium2 Bass kernel for nn_Criterion_32830730011569 (v2).

Strategy: 8 cores = (image b in 0..3) x (H-half h in 0..1). Each core streams
ONE concatenated [NPIX, 264] tensor (por|true|occ|occt_onehot) from HBM in 8
chunks; per chunk:
  - ACT: exp(por)->bf16, true->bf16 cast, exp(occ)
  - DVE: per-pixel-column fused mask-mult + softmax-denominator sum
    (scalar_tensor_tensor accum_out), reciprocal, a = true * (1/Z) in bf16,
    occupancy-CE partial reductions
  - PE : C[e,q] += a^T @ expm accumulated over all 144 pixel-columns
Window BCE uses per-element indirect DMA gathers (96x49 offsets computed on
host from the small index tensors), ~19KB each instead of multi-MB row runs.
ln/softplus use the ACT natural_log_exp table (Ln+Exp, single table load).
Each core returns 8 partial sums; the host combines them into the scalar loss.
"""
import sys

sys.path.insert(0, "/opt/trn_rl_repo")
import numpy as np

B, H, W, Q, E, M, K, WIN = 4, 192, 192, 160, 96, 96, 4, 7
NO_E = 0.1
HALF = H // 2          # rows per core slice
NPIX = HALF * W        # 18432 pixels per slice
P = 128                # partitions
J = NPIX // P          # 144 pixel-columns per partition (p-major)
NCHUNK = 8
JC = J // NCHUNK       # 18
SC = Q + E + K + K     # 264 stream columns: por | true | occ | occt_onehot
W49 = WIN * WIN

# sm32 pack column layout
C_OFFT = 0            # 49 int32 (bitcast) true-window element offsets into stream
C_OFFB = 49           # 49 int32 bin-window element offsets into bin_sl
C_VAL = 98            # 49 f32 window validity
C_HS = 147            # 160 f32 matched-pair indicator H[e, q]
C_IEL = 307           # 2 f32 class logits (packed 96x2, 160 used)
C_W = 309             # 2 f32 class weights
C_LAB = 311           # 2 f32 class labels
C_PTS = 313           # 2 f32 matched incidence points
C_CEN = 315           # 2 f32 matched centers
C_CHOL = 317          # 4 f32 matched chol (l00, l11, l10, pad)
C_IND = 321           # 160 f32 matched-q indicator row (replicated all parts)
NS = C_IND + Q        # 481

_CACHE = {}


def _build_nc():
    import concourse.bass as bass
    import concourse.bacc as bacc
    import concourse.tile as tile
    from concourse import mybir

    f32 = mybir.dt.float32
    i32 = mybir.dt.int32
    bf16 = mybir.dt.bfloat16
    AF = mybir.ActivationFunctionType
    OP = mybir.AluOpType
    AX = mybir.AxisListType

    nc = bacc.Bacc("TRN2", target_bir_lowering=False, debug=False, num_devices=8)

    stream = nc.dram_tensor("stream", [NPIX, SC], f32, kind="ExternalInput")
    bin_sl = nc.dram_tensor("bin_sl", [NPIX, Q], f32, kind="ExternalInput")
    sm32 = nc.dram_tensor("sm32", [P, NS], f32, kind="ExternalInput")
    partials = nc.dram_tensor("partials", [1, 8], f32, kind="ExternalOutput")

    def bc(ap, pos, count):
        """Insert a stride-0 broadcast dim into an AP at free-dim position pos."""
        new = list(ap.ap)
        new.insert(pos, [0, count])
        return bass.AP(tensor=ap.tensor, offset=ap.offset, ap=new)

    from contextlib import ExitStack

    with tile.TileContext(nc) as tc, ExitStack() as ctx:
        sing = ctx.enter_context(tc.tile_pool(name="sing", bufs=1))
        spool = ctx.enter_context(tc.tile_pool(name="spool", bufs=3))
        epool = ctx.enter_context(tc.tile_pool(name="epool", bufs=2))
        ps = ctx.enter_context(tc.tile_pool(name="ps", bufs=1, space="PSUM"))

        # ---------- small load ----------
        smt = sing.tile([P, NS], f32)
        nc.sync.dma_start(out=smt[:], in_=sm32.ap())

        # ---------- window gathers: per-element indirect DMA ----------
        tvw = sing.tile([M, W49], f32)
        bww = sing.tile([M, W49], f32)
        stream_flat = bass.AP(tensor=stream.ap().tensor, offset=0,
                              ap=[[1, NPIX * SC], [1, 1]])
        bin_flat = bass.AP(tensor=bin_sl.ap().tensor, offset=0,
                           ap=[[1, NPIX * Q], [1, 1]])
        offT = smt[0:M, C_OFFT:C_OFFT + W49].bitcast(i32)
        offB = smt[0:M, C_OFFB:C_OFFB + W49].bitcast(i32)
        nc.gpsimd.indirect_dma_start(
            out=tvw[:], out_offset=None, in_=stream_flat,
            in_offset=bass.IndirectOffsetOnAxis(ap=offT, axis=0))
        nc.gpsimd.indirect_dma_start(
            out=bww[:], out_offset=None, in_=bin_flat,
            in_offset=bass.IndirectOffsetOnAxis(ap=offB, axis=0))

        # matched-q indicator -> bf16 once
        indb = sing.tile([P, Q], bf16)
        nc.scalar.activation(out=indb[:], in_=smt[:, C_IND:C_IND + Q], func=AF.Copy)

        stats = sing.tile([P, 8], f32)
        nc.vector.memset(stats[:], 0.0)
        ones = sing.tile([P, 1], f32)
        nc.vector.memset(ones[:], 1.0)
        s4buf = sing.tile([P, J], f32)
        xtbuf = sing.tile([P, J], f32)
        res = sing.tile([1, 8], f32)
        nc.vector.memset(res[:], 0.0)

        # ---------- streamed dice + occupancy ----------
        stream_v = stream.ap().rearrange("(p j) c -> p j c", p=P)
        C_ps = ps.tile([E, Q], f32)

        def emit_side_work():
            # ---- window BCE ----
            ebw = sing.tile([M, W49], f32)
            nc.scalar.activation(out=ebw[:], in_=bww[:], func=AF.Exp)
            spw = sing.tile([M, W49], f32)
            nc.scalar.activation(out=spw[:], in_=ebw[:], func=AF.Ln, bias=1.0)
            prw = sing.tile([M, W49], f32)
            nc.vector.tensor_tensor(out=prw[:], in0=bww[:], in1=tvw[:], op=OP.mult)
            df = sing.tile([M, W49], f32)
            nc.vector.tensor_tensor(out=df[:], in0=spw[:], in1=prw[:], op=OP.subtract)
            scrw = sing.tile([M, W49], f32)
            nc.vector.scalar_tensor_tensor(
                out=scrw[:], in0=df[:], scalar=1.0, in1=smt[0:M, C_VAL:C_VAL + W49],
                op0=OP.mult, op1=OP.mult, accum_out=stats[0:M, 1:2])
            # ---- class loss ----
            ebc = sing.tile([M, 2], f32)
            nc.scalar.activation(out=ebc[:], in_=smt[0:M, C_IEL:C_IEL + 2], func=AF.Exp)
            spc = sing.tile([M, 2], f32)
            nc.scalar.activation(out=spc[:], in_=ebc[:], func=AF.Ln, bias=1.0)
            tc1 = sing.tile([M, 2], f32)
            nc.vector.scalar_tensor_tensor(
                out=tc1[:], in0=spc[:], scalar=1.0, in1=smt[0:M, C_W:C_W + 2],
                op0=OP.mult, op1=OP.mult, accum_out=stats[0:M, 6:7])
            tc2 = sing.tile([M, 2], f32)
            nc.vector.scalar_tensor_tensor(
                out=tc2[:], in0=smt[0:M, C_IEL:C_IEL + 2], scalar=1.0,
                in1=smt[0:M, C_LAB:C_LAB + 2],
                op0=OP.mult, op1=OP.mult, accum_out=stats[0:M, 7:8])
            # ---- NLL ----
            d2 = sing.tile([M, 2], f32)
            nc.vector.tensor_tensor(out=d2[:], in0=smt[0:M, C_PTS:C_PTS + 2],
                                    in1=smt[0:M, C_CEN:C_CEN + 2], op=OP.subtract)
            rr = sing.tile([M, 2], f32)
            nc.vector.reciprocal(out=rr[:], in_=smt[0:M, C_CHOL:C_CHOL + 2])
            zz = sing.tile([M, 2], f32)
            nc.vector.tensor_tensor(out=zz[:, 0:1], in0=d2[:, 0:1], in1=rr[:, 0:1], op=OP.mult)
            t1 = sing.tile([M, 1], f32)
            nc.vector.tensor_tensor(out=t1[:], in0=smt[0:M, C_CHOL + 2:C_CHOL + 3],
                                    in1=zz[:, 0:1], op=OP.mult)
            nc.vector.tensor_tensor(out=t1[:], in0=d2[:, 1:2], in1=t1[:], op=OP.subtract)
            nc.vector.tensor_tensor(out=zz[:, 1:2], in0=t1[:], in1=rr[:, 1:2], op=OP.mult)
            sqs = sing.tile([M, 2], f32)
            nc.vector.scalar_tensor_tensor(
                out=sqs[:], in0=zz[:], scalar=1.0, in1=zz[:],
                op0=OP.mult, op1=OP.mult, accum_out=stats[0:M, 5:6])
            ldet = sing.tile([M, 1], f32)
            nc.vector.tensor_tensor(out=ldet[:], in0=smt[0:M, C_CHOL:C_CHOL + 1],
                                    in1=smt[0:M, C_CHOL + 1:C_CHOL + 2], op=OP.mult)
            lnd = sing.tile([M, 1], f32)
            nc.scalar.activation(out=lnd[:], in_=ldet[:], func=AF.Ln)
            hq = sing.tile([M, 1], f32)
            nc.vector.tensor_scalar(out=hq[:], in0=stats[0:M, 5:6], scalar1=0.5,
                                    scalar2=float(np.log(2.0 * np.pi)),
                                    op0=OP.mult, op1=OP.add)
            nc.vector.tensor_tensor(out=stats[0:M, 0:1], in0=hq[:], in1=lnd[:], op=OP.add)

        for c in range(NCHUNK):
            sl = slice(c * JC, (c + 1) * JC)
            st = spool.tile([P, JC, SC], f32, tag="stream")
            nc.sync.dma_start(out=st[:], in_=stream_v[:, sl, :])
            ex = epool.tile([P, JC, Q], bf16, tag="exp")
            nc.scalar.activation(out=ex[:], in_=st[:, :, 0:Q], func=AF.Exp)
            Z = epool.tile([P, JC], f32, tag="Z")
            for j in range(JC):
                nc.vector.scalar_tensor_tensor(
                    out=ex[:, j, :], in0=ex[:, j, :], scalar=1.0, in1=indb[:],
                    op0=OP.mult, op1=OP.mult, accum_out=Z[:, j:j + 1])
            rz = epool.tile([P, JC], bf16, tag="rz")
            with nc.allow_low_precision(reason="bf16 1/Z; dice tolerance is loose"):
                nc.vector.reciprocal(out=rz[:], in_=Z[:])
            tb = epool.tile([P, JC, E], bf16, tag="tb")
            nc.scalar.activation(out=tb[:], in_=st[:, :, Q:Q + E], func=AF.Copy)
            nc.vector.tensor_tensor(out=tb[:], in0=tb[:], in1=bc(rz[:], 2, E), op=OP.mult)
            for j in range(JC):
                nc.tensor.matmul(out=C_ps[:], lhsT=tb[:, j, :], rhs=ex[:, j, :],
                                 start=(c == 0 and j == 0),
                                 stop=(c == NCHUNK - 1 and j == JC - 1))
            # occupancy CE partials
            eo = epool.tile([P, JC, K], f32, tag="eo")
            nc.scalar.activation(out=eo[:], in_=st[:, :, Q + E:Q + E + K], func=AF.Exp)
            nc.vector.tensor_reduce(out=s4buf[:, sl], in_=eo[:], axis=AX.X, op=OP.add)
            xo = epool.tile([P, JC, K], f32, tag="xo")
            nc.vector.tensor_tensor(out=xo[:], in0=st[:, :, Q + E:Q + E + K],
                                    in1=st[:, :, Q + E + K:SC], op=OP.mult)
            nc.vector.tensor_reduce(out=xtbuf[:, sl], in_=xo[:], axis=AX.X, op=OP.add)
            if c == 2:
                emit_side_work()

        # ---------- occupancy CE final ----------
        lse = sing.tile([P, J], f32)
        nc.scalar.activation(out=lse[:], in_=s4buf[:], func=AF.Ln)
        doc = sing.tile([P, J], f32)
        nc.vector.scalar_tensor_tensor(
            out=doc[:], in0=lse[:], scalar=1.0, in1=xtbuf[:],
            op0=OP.mult, op1=OP.subtract, accum_out=stats[:, 4:5])

        # ---------- dice final ----------
        scr2 = sing.tile([E, Q], f32)
        nc.vector.scalar_tensor_tensor(
            out=scr2[:], in0=C_ps[:], scalar=1.0, in1=smt[0:E, C_HS:C_HS + Q],
            op0=OP.mult, op1=OP.mult, accum_out=stats[0:E, 2:3])
        nc.vector.tensor_reduce(out=stats[0:E, 3:4], in_=C_ps[:], axis=AX.X, op=OP.add)

        # ---------- final cross-partition reduction ----------
        fin_ps = ps.tile([1, 8], f32)
        nc.tensor.matmul(out=fin_ps[:], lhsT=ones[:], rhs=stats[:], start=True, stop=True)
        nc.vector.tensor_copy(out=res[:], in_=fin_ps[:])
        nc.sync.dma_start(out=partials.ap(), in_=res[:])

    nc.compile()
    return nc


def _get_nc():
    if "nc" not in _CACHE:
        _CACHE["nc"] = _build_nc()
    return _CACHE["nc"]


def make_in_maps(is_electron_logit, true_segmap, binary_mask_logits, portion_logits,
                 incidence_points, positions, chol, occupancy_logits, occupancy_true,
                 matched_q, matched_e):
    f = np.float32
    eye4 = np.eye(K, dtype=f)
    dr7 = np.arange(WIN) - WIN // 2
    in_maps = []
    for c in range(8):
        b, h = c // 2, c % 2
        sl = slice(h * HALF, (h + 1) * HALF)
        me = np.asarray(matched_e[b]).astype(np.int64)
        mq = np.asarray(matched_q[b]).astype(np.int64)

        por = np.asarray(portion_logits[b, sl], dtype=f).reshape(NPIX, Q)
        tru = np.asarray(true_segmap[b, sl], dtype=f).reshape(NPIX, E)
        occ = np.asarray(occupancy_logits[b, sl], dtype=f).reshape(NPIX, K)
        occt = np.asarray(occupancy_true[b, sl]).reshape(NPIX)
        stream = np.concatenate([por, tru, occ, eye4[occt]], axis=1)

        # window offsets / validity (from small tensors only)
        pts = np.asarray(incidence_points[b], dtype=f)[me]          # [M,2]
        pix = np.floor(pts).astype(np.int64)
        rg = pix[:, 0:1] + dr7[None, :]                              # [M,7] global rows
        cg = pix[:, 1:2] + dr7[None, :]                              # [M,7] cols
        valid = ((rg >= h * HALF) & (rg < (h + 1) * HALF)).astype(f)  # [M,7]
        rl = rg - h * HALF
        flat = rl[:, :, None] * W + cg[:, None, :]                   # [M,7,7]
        flat = np.clip(flat, 0, NPIX - 1)
        offT = (flat * SC + Q + me[:, None, None]).astype(np.int32).reshape(M, W49)
        offB = (flat * Q + mq[:, None, None]).astype(np.int32).reshape(M, W49)
        valid49 = np.broadcast_to(valid[:, :, None], (M, WIN, WIN)).reshape(M, W49)

        # matched-pair indicator H[e,q], matched-q indicator row
        Hs = np.zeros((E, Q), dtype=f)
        Hs[me, mq] = 1.0
        ind = np.zeros(Q, dtype=f)
        ind[mq] = 1.0

        # class-loss packing: slot s = p + 96*col, 160 used of 192
        iel = np.asarray(is_electron_logit, dtype=f).reshape(B, Q)[b]
        lab = np.zeros(Q, dtype=f)
        lab[mq] = 1.0
        wgt = NO_E + (1.0 - NO_E) * lab
        def pack2(v):
            tmp = np.zeros(2 * M, dtype=f)
            tmp[:Q] = v
            return np.ascontiguousarray(tmp.reshape(2, M).T)

        iel2, lab2, w2 = pack2(iel), pack2(lab), pack2(wgt)

        chol_b = np.asarray(chol[b], dtype=f)[mq]                    # [M,2,2]
        cen = np.asarray(positions[b], dtype=f)[mq]                  # [M,2]

        sm = np.zeros((P, NS), dtype=f)
        sm[0:M, C_OFFT:C_OFFT + W49] = offT.view(f)
        sm[0:M, C_OFFB:C_OFFB + W49] = offB.view(f)
        sm[0:M, C_VAL:C_VAL + W49] = valid49
        sm[0:E, C_HS:C_HS + Q] = Hs
        sm[0:M, C_IEL:C_IEL + 2] = iel2
        sm[0:M, C_W:C_W + 2] = w2
        sm[0:M, C_LAB:C_LAB + 2] = lab2
        sm[0:M, C_PTS:C_PTS + 2] = pts
        sm[0:M, C_CEN:C_CEN + 2] = cen
        sm[0:M, C_CHOL + 0] = chol_b[:, 0, 0]
        sm[0:M, C_CHOL + 1] = chol_b[:, 1, 1]
        sm[0:M, C_CHOL + 2] = chol_b[:, 1, 0]
        sm[:, C_IND:C_IND + Q] = ind[None, :]

        in_maps.append(dict(
            stream=np.ascontiguousarray(stream),
            bin_sl=np.ascontiguousarray(binary_mask_logits[b, sl]).reshape(NPIX, Q),
            sm32=sm,
        ))
    return in_maps


def combine(partials_list):
    s = np.stack([np.asarray(p, dtype=np.float64).reshape(8) for p in partials_list])
    # slots: 0=nll_sum 1=bce_sum 2=num2_sum 3=den_true_sum 4=occ_sum 6=cls_sp 7=cls_xz
    class_loss = (s[0::2, 6].sum() - s[0::2, 7].sum()) / (B * Q)
    nll_loss = s[0::2, 0].sum() / (B * M)
    bce_loss = s[:, 1].sum() / (B * M * W49)
    occ_loss = s[:, 4].sum() / (B * H * W)
    dice = 0.0
    for b in range(B):
        num = 2.0 * (s[2 * b, 2] + s[2 * b + 1, 2])
        den = s[2 * b, 3] + s[2 * b + 1, 3] + H * W
        dice += 1.0 - (num + 1.0) / (den + 1.0)
    dice_loss = dice / B
    return np.float32(class_loss + bce_loss + dice_loss + nll_loss + occ_loss)


def kernel(**inputs):
    from concourse.bass_utils import run_bass_kernel_spmd
    nc = _get_nc()
    in_maps = make_in_maps(**{k: np.asarray(v) for k, v in inputs.items()})
    r = run_bass_kernel_spmd(nc, in_maps, list(range(8)))
    return combine([r.results[c]["partials"] for c in range(8)])


# revision 3
# speedup vs baseline: 1.4109x; 1.0500x over previous
"""Trainium2 Bass kernel for nn_Criterion_32830730011569 (v3).

8 cores = (image b) x (H-half h). Each core streams ONE concatenated
[NPIX, 264] f32 tensor (por|true|occ|occt_onehot) in 9 chunks (small first
chunk to cut lead-in). Per chunk:
  ACT: exp(por)->bf16, true->bf16 cast with accum_out (= dice den partial),
       exp(occ)
  DVE: bf16 mask-mult, bf16 tree-sum for the softmax denominator Z,
       reciprocal, a = true*(1/Z), occ·onehot product (bf16)
  PE : C[0:96,q] += a^T @ [expm | occ·onehot]; ones column in the stationary
       makes row 96 the pixel-sum row -> occupancy x-term for free.
Window BCE gathers 96x49 elements via 2 indirect DMAs (host-computed offsets).
ln/softplus on ACT (Ln table), all Ln uses batched at the end (2 table loads).
"""
import sys

sys.path.insert(0, "/opt/trn_rl_repo")
import numpy as np

B, H, W, Q, E, M, K, WIN = 4, 192, 192, 160, 96, 96, 4, 7
NO_E = 0.1
HALF = H // 2
NPIX = HALF * W        # 18432
P = 128
J = NPIX // P          # 144
JCS = [6] + [18] * 7 + [12]          # per-chunk pixel-columns, sums to 144
SC = Q + E + K + K     # 264 stream cols: por | true | occ | occt_onehot
W49 = WIN * WIN
RQ = Q + K             # 164 rhs cols: expm | occ*onehot
LE = E + 1             # 97 stationary cols: true*rz | ones

# sm32 pack column layout
C_OFFT = 0
C_OFFB = 49
C_VAL = 98
C_HS = 147
C_IEL = 307
C_W = 309
C_LAB = 311
C_PTS = 313
C_CEN = 315
C_CHOL = 317
C_IND = 321
NS = C_IND + Q         # 481

_CACHE = {}


def _build_nc():
    import concourse.bass as bass
    import concourse.bacc as bacc
    import concourse.tile as tile
    from concourse import mybir

    f32 = mybir.dt.float32
    i32 = mybir.dt.int32
    bf16 = mybir.dt.bfloat16
    AF = mybir.ActivationFunctionType
    OP = mybir.AluOpType
    AX = mybir.AxisListType

    nc = bacc.Bacc("TRN2", target_bir_lowering=False, debug=False, num_devices=8)

    stream = nc.dram_tensor("stream", [NPIX, SC], f32, kind="ExternalInput")
    bin_sl = nc.dram_tensor("bin_sl", [NPIX, Q], f32, kind="ExternalInput")
    sm32 = nc.dram_tensor("sm32", [P, NS], f32, kind="ExternalInput")
    partials = nc.dram_tensor("partials", [1, 12], f32, kind="ExternalOutput")

    def bc(ap, pos, count):
        new = list(ap.ap)
        new.insert(pos, [0, count])
        return bass.AP(tensor=ap.tensor, offset=ap.offset, ap=new)

    from contextlib import ExitStack

    with tile.TileContext(nc) as tc, ExitStack() as ctx:
        sing = ctx.enter_context(tc.tile_pool(name="sing", bufs=1))
        spool = ctx.enter_context(tc.tile_pool(name="spool", bufs=3))
        epool = ctx.enter_context(tc.tile_pool(name="epool", bufs=2))
        ps = ctx.enter_context(tc.tile_pool(name="ps", bufs=1, space="PSUM"))

        smt = sing.tile([P, NS], f32)
        nc.sync.dma_start(out=smt[:], in_=sm32.ap())

        # window gathers: per-element indirect DMA (offsets from host)
        tvw = sing.tile([M, W49], f32)
        bww = sing.tile([M, W49], f32)
        stream_flat = bass.AP(tensor=stream.ap().tensor, offset=0,
                              ap=[[1, NPIX * SC], [1, 1]])
        bin_flat = bass.AP(tensor=bin_sl.ap().tensor, offset=0,
                           ap=[[1, NPIX * Q], [1, 1]])
        offT = smt[0:M, C_OFFT:C_OFFT + W49].bitcast(i32)
        offB = smt[0:M, C_OFFB:C_OFFB + W49].bitcast(i32)
        nc.gpsimd.indirect_dma_start(
            out=tvw[:], out_offset=None, in_=stream_flat,
            in_offset=bass.IndirectOffsetOnAxis(ap=offT, axis=0))
        nc.gpsimd.indirect_dma_start(
            out=bww[:], out_offset=None, in_=bin_flat,
            in_offset=bass.IndirectOffsetOnAxis(ap=offB, axis=0))

        indb = sing.tile([P, Q], bf16)
        nc.scalar.activation(out=indb[:], in_=smt[:, C_IND:C_IND + Q], func=AF.Copy)

        stats = sing.tile([P, 12], f32)
        nc.vector.memset(stats[:], 0.0)
        ones = sing.tile([P, 1], f32)
        nc.vector.memset(ones[:], 1.0)
        s4buf = sing.tile([P, J], f32)
        den_acc = sing.tile([P, len(JCS)], f32)
        res = sing.tile([1, 12], f32)
        nc.vector.memset(res[:], 0.0)

        stream_v = stream.ap().rearrange("(p j) c -> p j c", p=P)
        C_ps = ps.tile([LE, RQ], f32)

        # mid-loop side work: only Exp-table activations + DVE ops
        def emit_side_exp():
            ebw = sing.tile([M, W49], f32)
            nc.scalar.activation(out=ebw[:], in_=bww[:], func=AF.Exp)
            ebc = sing.tile([M, 2], f32)
            nc.scalar.activation(out=ebc[:], in_=smt[0:M, C_IEL:C_IEL + 2], func=AF.Exp)
            prw = sing.tile([M, W49], f32)
            nc.vector.tensor_tensor(out=prw[:], in0=bww[:], in1=tvw[:], op=OP.mult)
            # NLL DVE part (no ln yet)
            d2 = sing.tile([M, 2], f32)
            nc.vector.tensor_tensor(out=d2[:], in0=smt[0:M, C_PTS:C_PTS + 2],
                                    in1=smt[0:M, C_CEN:C_CEN + 2], op=OP.subtract)
            rr = sing.tile([M, 2], f32)
            nc.vector.reciprocal(out=rr[:], in_=smt[0:M, C_CHOL:C_CHOL + 2])
            zz = sing.tile([M, 2], f32)
            nc.vector.tensor_tensor(out=zz[:, 0:1], in0=d2[:, 0:1], in1=rr[:, 0:1], op=OP.mult)
            t1 = sing.tile([M, 1], f32)
            nc.vector.tensor_tensor(out=t1[:], in0=smt[0:M, C_CHOL + 2:C_CHOL + 3],
                                    in1=zz[:, 0:1], op=OP.mult)
            nc.vector.tensor_tensor(out=t1[:], in0=d2[:, 1:2], in1=t1[:], op=OP.subtract)
            nc.vector.tensor_tensor(out=zz[:, 1:2], in0=t1[:], in1=rr[:, 1:2], op=OP.mult)
            sqs = sing.tile([M, 2], f32)
            nc.vector.scalar_tensor_tensor(
                out=sqs[:], in0=zz[:], scalar=1.0, in1=zz[:],
                op0=OP.mult, op1=OP.mult, accum_out=stats[0:M, 8:9])
            ldet = sing.tile([M, 1], f32)
            nc.vector.tensor_tensor(out=ldet[:], in0=smt[0:M, C_CHOL:C_CHOL + 1],
                                    in1=smt[0:M, C_CHOL + 1:C_CHOL + 2], op=OP.mult)
            return ebw, ebc, prw, ldet

        side = {}
        JMAX = max(JCS)
        j0 = 0
        for c, JC in enumerate(JCS):
            sl = slice(j0, j0 + JC)
            j0 += JC
            stf = spool.tile([P, JMAX, SC], f32, tag="stream")
            st = stf[:, 0:JC, :]
            nc.sync.dma_start(out=st, in_=stream_v[:, sl, :])
            exf = epool.tile([P, JMAX, RQ], bf16, tag="exp")
            ex = exf[:, 0:JC, :]
            nc.scalar.activation(out=ex[:, :, 0:Q], in_=st[:, :, 0:Q], func=AF.Exp)
            # masked expm (in-place), matmul rhs cols 0:160
            nc.vector.tensor_tensor(out=ex[:, :, 0:Q], in0=ex[:, :, 0:Q],
                                    in1=bc(indb[:], 1, JC), op=OP.mult)
            # occ*onehot -> rhs cols 160:164
            nc.vector.tensor_tensor(out=ex[:, :, Q:RQ], in0=st[:, :, Q + E:Q + E + K],
                                    in1=st[:, :, Q + E + K:SC], op=OP.mult)
            # bf16 tree-sum of masked expm -> Z
            scrf = epool.tile([P, JMAX, Q // 2], bf16, tag="scr")
            scr = scrf[:, 0:JC, :]
            w_ = Q // 2
            nc.vector.tensor_tensor(out=scr[:, :, 0:w_], in0=ex[:, :, 0:w_],
                                    in1=ex[:, :, w_:Q], op=OP.add)
            while w_ > 5:
                h_ = w_ // 2
                nc.vector.tensor_tensor(out=scr[:, :, 0:h_], in0=scr[:, :, 0:h_],
                                        in1=scr[:, :, h_:w_], op=OP.add)
                w_ = h_
            Zf = epool.tile([P, JMAX], f32, tag="Z")
            Z = Zf[:, 0:JC]
            nc.vector.tensor_reduce(out=Z, in_=scr[:, :, 0:w_], axis=AX.X, op=OP.add)
            rzf = epool.tile([P, JMAX], bf16, tag="rz")
            rz = rzf[:, 0:JC]
            with nc.allow_low_precision(reason="bf16 1/Z; dice tolerance is loose"):
                nc.vector.reciprocal(out=rz, in_=Z)
            tbf = epool.tile([P, JMAX, LE], bf16, tag="tb")
            tb = tbf[:, 0:JC, :]
            nc.scalar.activation(out=tb[:, :, 0:E], in_=st[:, :, Q:Q + E], func=AF.Copy,
                                 accum_out=den_acc[:, c:c + 1])
            nc.vector.tensor_tensor(out=tb[:, :, 0:E], in0=tb[:, :, 0:E],
                                    in1=bc(rz, 2, E), op=OP.mult)
            nc.vector.memset(tb[:, :, E:LE], 1.0)
            for j in range(JC):
                nc.tensor.matmul(out=C_ps[:], lhsT=tb[:, j, :], rhs=ex[:, j, :],
                                 start=(c == 0 and j == 0),
                                 stop=(c == len(JCS) - 1 and j == JC - 1))
            # occ logsumexp partial: s4 = sum_k exp(occ)
            eof = epool.tile([P, JMAX, K], f32, tag="eo")
            eo = eof[:, 0:JC, :]
            nc.scalar.activation(out=eo, in_=st[:, :, Q + E:Q + E + K], func=AF.Exp)
            nc.vector.tensor_reduce(out=s4buf[:, sl], in_=eo, axis=AX.X, op=OP.add)
            if c == 2:
                side.update(zip(("ebw", "ebc", "prw", "ldet"), emit_side_exp()))

        # ---------- tail: all Ln-table work ----------
        # occ logsumexp: sum_j ln(s4) per partition via ACT accumulator
        lse = sing.tile([P, J], f32)
        nc.scalar.activation(out=lse[:], in_=s4buf[:], func=AF.Ln,
                             accum_out=stats[:, 4:5])
        # window BCE: softplus = ln(exp+1)
        spw = sing.tile([M, W49], f32)
        nc.scalar.activation(out=spw[:], in_=side["ebw"][:], func=AF.Ln, bias=1.0)
        dfw = sing.tile([M, W49], f32)
        nc.vector.tensor_tensor(out=dfw[:], in0=spw[:], in1=side["prw"][:], op=OP.subtract)
        scrw = sing.tile([M, W49], f32)
        nc.vector.scalar_tensor_tensor(
            out=scrw[:], in0=dfw[:], scalar=1.0, in1=smt[0:M, C_VAL:C_VAL + W49],
            op0=OP.mult, op1=OP.mult, accum_out=stats[0:M, 1:2])
        # class loss
        spc = sing.tile([M, 2], f32)
        nc.scalar.activation(out=spc[:], in_=side["ebc"][:], func=AF.Ln, bias=1.0)
        tc1 = sing.tile([M, 2], f32)
        nc.vector.scalar_tensor_tensor(
            out=tc1[:], in0=spc[:], scalar=1.0, in1=smt[0:M, C_W:C_W + 2],
            op0=OP.mult, op1=OP.mult, accum_out=stats[0:M, 6:7])
        tc2 = sing.tile([M, 2], f32)
        nc.vector.scalar_tensor_tensor(
            out=tc2[:], in0=smt[0:M, C_IEL:C_IEL + 2], scalar=1.0,
            in1=smt[0:M, C_LAB:C_LAB + 2],
            op0=OP.mult, op1=OP.mult, accum_out=stats[0:M, 7:8])
        # NLL: 0.5*(z0^2+z1^2) + log(2pi) + ln(l00*l11)
        lnd = sing.tile([M, 1], f32)
        nc.scalar.activation(out=lnd[:], in_=side["ldet"][:], func=AF.Ln)
        hq = sing.tile([M, 1], f32)
        nc.vector.tensor_scalar(out=hq[:], in0=stats[0:M, 8:9], scalar1=0.5,
                                scalar2=float(np.log(2.0 * np.pi)),
                                op0=OP.mult, op1=OP.add)
        nc.vector.tensor_tensor(out=stats[0:M, 0:1], in0=hq[:], in1=lnd[:], op=OP.add)
        # dice den partials
        nc.vector.tensor_reduce(out=stats[:, 3:4], in_=den_acc[:], axis=AX.X, op=OP.add)
        # dice num: sum over matched pairs of C
        scr2 = sing.tile([E, Q], f32)
        nc.vector.scalar_tensor_tensor(
            out=scr2[:], in0=C_ps[0:E, 0:Q], scalar=1.0, in1=smt[0:E, C_HS:C_HS + Q],
            op0=OP.mult, op1=OP.mult, accum_out=stats[0:E, 2:3])
        # occ x-term total: C row 96, cols 160:164 -> stats[96, 5]
        nc.vector.tensor_reduce(out=stats[E:LE, 5:6], in_=C_ps[E:LE, Q:RQ],
                                axis=AX.X, op=OP.add)

        fin_ps = ps.tile([1, 12], f32)
        nc.tensor.matmul(out=fin_ps[:], lhsT=ones[:], rhs=stats[:], start=True, stop=True)
        nc.vector.tensor_copy(out=res[:], in_=fin_ps[:])
        nc.sync.dma_start(out=partials.ap(), in_=res[:])

    nc.compile()
    return nc


def _get_nc():
    if "nc" not in _CACHE:
        _CACHE["nc"] = _build_nc()
    return _CACHE["nc"]


def make_in_maps(is_electron_logit, true_segmap, binary_mask_logits, portion_logits,
                 incidence_points, positions, chol, occupancy_logits, occupancy_true,
                 matched_q, matched_e):
    f = np.float32
    eye4 = np.eye(K, dtype=f)
    dr7 = np.arange(WIN) - WIN // 2
    in_maps = []
    for c in range(8):
        b, h = c // 2, c % 2
        sl = slice(h * HALF, (h + 1) * HALF)
        me = np.asarray(matched_e[b]).astype(np.int64)
        mq = np.asarray(matched_q[b]).astype(np.int64)

        por = np.asarray(portion_logits[b, sl], dtype=f).reshape(NPIX, Q)
        tru = np.asarray(true_segmap[b, sl], dtype=f).reshape(NPIX, E)
        occ = np.asarray(occupancy_logits[b, sl], dtype=f).reshape(NPIX, K)
        occt = np.asarray(occupancy_true[b, sl]).reshape(NPIX)
        stream = np.concatenate([por, tru, occ, eye4[occt]], axis=1)

        pts = np.asarray(incidence_points[b], dtype=f)[me]
        pix = np.floor(pts).astype(np.int64)
        rg = pix[:, 0:1] + dr7[None, :]
        cg = pix[:, 1:2] + dr7[None, :]
        valid = ((rg >= h * HALF) & (rg < (h + 1) * HALF)).astype(f)
        rl = rg - h * HALF
        flat = rl[:, :, None] * W + cg[:, None, :]
        flat = np.clip(flat, 0, NPIX - 1)
        offT = (flat * SC + Q + me[:, None, None]).astype(np.int32).reshape(M, W49)
        offB = (flat * Q + mq[:, None, None]).astype(np.int32).reshape(M, W49)
        valid49 = np.ascontiguousarray(
            np.broadcast_to(valid[:, :, None], (M, WIN, WIN))).reshape(M, W49)

        Hs = np.zeros((E, Q), dtype=f)
        Hs[me, mq] = 1.0
        ind = np.zeros(Q, dtype=f)
        ind[mq] = 1.0

        iel = np.asarray(is_electron_logit, dtype=f).reshape(B, Q)[b]
        lab = np.zeros(Q, dtype=f)
        lab[mq] = 1.0
        wgt = NO_E + (1.0 - NO_E) * lab

        def pack2(v):
            tmp = np.zeros(2 * M, dtype=f)
            tmp[:Q] = v
            return np.ascontiguousarray(tmp.reshape(2, M).T)

        iel2, lab2, w2 = pack2(iel), pack2(lab), pack2(wgt)

        chol_b = np.asarray(chol[b], dtype=f)[mq]
        cen = np.asarray(positions[b], dtype=f)[mq]

        sm = np.zeros((P, NS), dtype=f)
        sm[0:M, C_OFFT:C_OFFT + W49] = offT.view(f)
        sm[0:M, C_OFFB:C_OFFB + W49] = offB.view(f)
        sm[0:M, C_VAL:C_VAL + W49] = valid49
        sm[0:E, C_HS:C_HS + Q] = Hs
        sm[0:M, C_IEL:C_IEL + 2] = iel2
        sm[0:M, C_W:C_W + 2] = w2
        sm[0:M, C_LAB:C_LAB + 2] = lab2
        sm[0:M, C_PTS:C_PTS + 2] = pts
        sm[0:M, C_CEN:C_CEN + 2] = cen
        sm[0:M, C_CHOL + 0] = chol_b[:, 0, 0]
        sm[0:M, C_CHOL + 1] = chol_b[:, 1, 1]
        sm[0:M, C_CHOL + 2] = chol_b[:, 1, 0]
        sm[:, C_IND:C_IND + Q] = ind[None, :]

        in_maps.append(dict(
            stream=np.ascontiguousarray(stream),
            bin_sl=np.ascontiguousarray(binary_mask_logits[b, sl]).reshape(NPIX, Q),
            sm32=sm,
        ))
    return in_maps


def combine(partials_list):
    s = np.stack([np.asarray(p, dtype=np.float64).reshape(12) for p in partials_list])
    # slots: 0=nll 1=bce 2=num2 3=den_true 4=sum_lse 5=occ_xt 6=cls_sp 7=cls_xz
    class_loss = (s[0::2, 6].sum() - s[0::2, 7].sum()) / (B * Q)
    nll_loss = s[0::2, 0].sum() / (B * M)
    bce_loss = s[:, 1].sum() / (B * M * W49)
    occ_loss = (s[:, 4].sum() - s[:, 5].sum()) / (B * H * W)
    dice = 0.0
    for b in range(B):
        num = 2.0 * (s[2 * b, 2] + s[2 * b + 1, 2])
        den = s[2 * b, 3] + s[2 * b + 1, 3] + H * W
        dice += 1.0 - (num + 1.0) / (den + 1.0)
    dice_loss = dice / B
    return np.float32(class_loss + bce_loss + dice_loss + nll_loss + occ_loss)


def kernel(**inputs):
    from concourse.bass_utils import run_bass_kernel_spmd
    nc = _get_nc()
    in_maps = make_in_maps(**{k: np.asarray(v) for k, v in inputs.items()})
    r = run_bass_kernel_spmd(nc, in_maps, list(range(8)))
    return combine([r.results[c]["partials"] for c in range(8)])
